# revision 11
# baseline (speedup 1.0000x reference)
"""Trainium2 kernel for per-subject linear heads (moe_routing).

Computes out[i] = x[i] @ W[subject_ids[i]] + b[subject_ids[i]] for
B=256, D=2048, S=8 subjects, OUT=1000.

Sharding: expert-parallel — core s owns subject s. Each core reads only
its own (2048, 1000) fp32 weight slice (8.19 MB) from HBM, so the total
weight traffic across the chip is W read exactly once (vs 8x for
batch-data-parallel with a replicated table). Samples are grouped by
subject on the host, padded to a fixed capacity C, and fed to an SPMD
Bass/Tile kernel; outputs are scattered back to the original order.

The bias is folded into the matmul accumulation as a rank-1 update:
a row of ones (carried as an extra k-slot of the x input) times the
[1, OUT] bias row opens each PSUM accumulation group. This keeps every
instruction at <= 1 semaphore wait — this walrus build rejects
instructions with more ("Too many sync wait commands").
"""

import numpy as np

import concourse.bass as bass
import concourse.mybir as mybir
import concourse.tile as tile
from concourse.bass_utils import run_bass_kernel_spmd
from concourse.vector_clock import ScopedClock, VectorClock


class _SingleWaitTileContext(tile.TileContext):
    """TileContext whose kernel-tail drain carries one semaphore wait per
    drain instruction.

    The walrus build in this container rejects any instruction with more
    than one sync wait ("Too many sync wait commands"), and the stock
    TileContext emits a single tail Drain waiting on every semaphore at
    once. Emitting one Drain per logical processor keeps each at a
    single wait; successive drains advance the SP engine's observed
    clock, so no wait is duplicated.
    """

    def _drain_and_barrier(self, tick_clock, wait_clock):
        gc = tick_clock.global_clock
        n = len(gc)
        for i in range(n):
            if gc[i] <= 0:
                continue
            vec = [0] * n
            vec[i] = gc[i]
            d = self.nc.sync.drain()
            wait_clock.add_sem_waits(d.ins, ScopedClock({None: VectorClock(vec)}))

        self.nc.all_engine_barrier()
        assert self.sems is not None
        popped = self.nc._tile_sem_poison_stack.pop()
        assert popped is self._sem_poison
        self.nc.clear_and_free_semaphores(list(self.sems.allocated().values()))
        self.nc.all_engine_barrier()

B = 256
D = 2048
S = 8
OUT = 1000
P = 128
KO = D // P          # 16 k-tiles of 128
NT = 500             # psum n-tile (<= 512 fp32 / bank), 2 tiles cover OUT
CH = 4               # k-tiles per W DMA chunk (4 * 128 * 1000 * 4B = 2 MB)

TRACE = False        # set by test harness to collect an NTFF profile
LAST_RESULTS = None  # BassKernelResults of the most recent run

_nc_cache = {}


def _build(C):
    """Per-core program: y[C, OUT] = xT.T @ w + bias, all fp32.

    xT   : [P, KO+1, C]  xT[p, ko, c] = x_subject[c, ko*P + p] for
                         ko < KO; the last slot is all-ones (bias row).
    w    : [D, OUT]      natural layout of one subject's weights
    bias : [1, OUT]      the subject's bias row
    """
    nc = bass.Bass()
    xT = nc.dram_tensor("xT", [P, KO + 1, C], mybir.dt.float32, kind="ExternalInput")
    w = nc.dram_tensor("w", [D, OUT], mybir.dt.float32, kind="ExternalInput")
    bias = nc.dram_tensor("bias", [1, OUT], mybir.dt.float32, kind="ExternalInput")
    y = nc.dram_tensor("y", [C, OUT], mybir.dt.float32, kind="ExternalOutput")

    w_t = w.rearrange("(ko p) n -> p ko n", p=P)  # [128, 16, 1000]
    n_chunks = KO // CH
    m_tiles = [(m0, min(P, C - m0)) for m0 in range(0, C, P)]

    with _SingleWaitTileContext(nc) as tc:
        with (
            tc.tile_pool(name="wpool", bufs=n_chunks) as wpool,
            tc.tile_pool(name="xpool", bufs=1) as xpool,
            tc.tile_pool(name="bpool", bufs=1) as bpool,
            tc.tile_pool(name="opool", bufs=4) as opool,
            tc.tile_pool(name="psum", bufs=1, space="PSUM") as psum_pool,
        ):
            # Exactly 8 DMAs total (x, bias, 4 w chunks, 2 y writes for
            # C <= 128) — one per HWDGE completion-sem lane, so no
            # instruction ever needs a lane-reuse wait.
            x_tile = xpool.tile([P, KO + 1, C], mybir.dt.float32)
            nc.sync.dma_start(x_tile[:], xT[:])
            b_tile = bpool.tile([1, OUT], mybir.dt.float32)
            nc.sync.dma_start(b_tile[:], bias[:])

            w_tiles = []
            for ch in range(n_chunks):
                wt = wpool.tile([P, CH, OUT], mybir.dt.float32)
                nc.sync.dma_start(wt[:], w_t[:, ch * CH : (ch + 1) * CH, :])
                w_tiles.append(wt)

            psums = {
                (mi, n): psum_pool.tile(
                    [mc, NT], mybir.dt.float32, name=f"psum_{mi}_{n}"
                )
                for mi, (m0, mc) in enumerate(m_tiles)
                for n in range(2)
            }
            # Each instruction has ONE wait slot. The first PE op below
            # absorbs the x-DMA wait (its lhsT is a preamble constant, so
            # it has no other dependency); the bias matmuls then only
            # wait on the bias DMA, and each k-chunk's first matmul only
            # on that chunk's DMA.
            warm = psum_pool.tile([1, C], mybir.dt.float32, name="warm")
            nc.tensor.matmul(
                warm[:, :],
                nc.const_aps.tensor(0.0, [P, 1], mybir.dt.float32),
                x_tile[:, 0, :],
                start=True,
                stop=True,
            )
            # Open each accumulation group with the rank-1 bias update:
            # ones[1, mc].T @ bias[1, NT].
            for mi, (m0, mc) in enumerate(m_tiles):
                for n in range(2):
                    nc.tensor.matmul(
                        psums[(mi, n)][:, :],
                        x_tile[0:1, KO, m0 : m0 + mc],
                        b_tile[0:1, n * NT : (n + 1) * NT],
                        start=True,
                        stop=False,
                    )
            # k-contiguous loop: each W chunk is consumed for every
            # (m, n) output tile as soon as it lands, then is dead.
            for ko in range(KO):
                wt = w_tiles[ko // CH]
                for mi, (m0, mc) in enumerate(m_tiles):
                    lhsT = x_tile[:, ko, m0 : m0 + mc]
                    for n in range(2):
                        nc.tensor.matmul(
                            psums[(mi, n)][:, :],
                            lhsT,
                            wt[:, ko % CH, n * NT : (n + 1) * NT],
                            start=False,
                            stop=(ko == KO - 1),
                        )
            for mi, (m0, mc) in enumerate(m_tiles):
                for n in range(2):
                    ot = opool.tile([mc, NT], mybir.dt.float32)
                    nc.vector.tensor_copy(ot[:], psums[(mi, n)][:])
                    nc.sync.dma_start(y[m0 : m0 + mc, n * NT : (n + 1) * NT], ot[:])
    return nc


def _capacity(max_count):
    c = 64
    while c < max_count:
        c *= 2
    return c


def kernel(x, subject_ids, W, b):
    global LAST_RESULTS
    x = np.ascontiguousarray(np.asarray(x, dtype=np.float32))
    sid = np.asarray(subject_ids).astype(np.int64)
    W = np.ascontiguousarray(np.asarray(W, dtype=np.float32))
    b = np.ascontiguousarray(np.asarray(b, dtype=np.float32))

    groups = [np.nonzero(sid == s)[0] for s in range(S)]
    C = _capacity(max((len(g) for g in groups), default=1))

    if C not in _nc_cache:
        _nc_cache[C] = _build(C)
    nc = _nc_cache[C]

    in_maps = []
    for s in range(S):
        idx = groups[s]
        xs = np.zeros((C, D), dtype=np.float32)
        xs[: len(idx)] = x[idx]
        # [p, ko, c] = xs[c, ko*P + p]; extra all-ones k-slot for bias
        xT = np.empty((P, KO + 1, C), dtype=np.float32)
        xT[:, :KO, :] = xs.T.reshape(KO, P, C).transpose(1, 0, 2)
        xT[:, KO, :] = 1.0
        in_maps.append({"xT": xT, "w": W[s], "bias": b[s : s + 1]})

    LAST_RESULTS = run_bass_kernel_spmd(
        nc, in_maps, core_ids=list(range(S)), trace=TRACE
    )

    out = np.zeros((B, OUT), dtype=np.float32)
    for s in range(S):
        idx = groups[s]
        out[idx] = LAST_RESULTS.results[s]["y"][: len(idx)]
    return out


# revision 14
# speedup vs baseline: 1.2699x; 1.2699x over previous
"""Trainium2 kernel for per-subject linear heads (moe_routing).

Computes out[i] = x[i] @ W[subject_ids[i]] + b[subject_ids[i]] for
B=256, D=2048, S=8 subjects, OUT=1000.

Sharding: expert-parallel — core s owns subject s. Each core reads only
its own (2048, 1000) fp32 weight slice (8.19 MB) from HBM, so the total
weight traffic across the chip is W read exactly once (vs 8x for
batch-data-parallel with a replicated table). Samples are grouped by
subject on the host, padded to a fixed capacity C, and fed to an SPMD
Bass/Tile kernel; outputs are scattered back to the original order.

The bias is folded into the matmul accumulation as a rank-1 update:
a row of ones (carried as an extra k-slot of the x input) times the
[1, OUT] bias row opens each PSUM accumulation group. This keeps every
instruction at <= 1 semaphore wait — this walrus build rejects
instructions with more ("Too many sync wait commands").
"""

import numpy as np

import concourse.bass as bass
import concourse.mybir as mybir
import concourse.tile as tile
from concourse.bass_utils import run_bass_kernel_spmd
from concourse.vector_clock import ScopedClock, VectorClock


class _SingleWaitTileContext(tile.TileContext):
    """TileContext whose kernel-tail drain carries one semaphore wait per
    drain instruction.

    The walrus build in this container rejects any instruction with more
    than one sync wait ("Too many sync wait commands"), and the stock
    TileContext emits a single tail Drain waiting on every semaphore at
    once. Emitting one Drain per logical processor keeps each at a
    single wait; successive drains advance the SP engine's observed
    clock, so no wait is duplicated.
    """

    def _drain_and_barrier(self, tick_clock, wait_clock):
        gc = tick_clock.global_clock
        n = len(gc)
        for i in range(n):
            if gc[i] <= 0:
                continue
            vec = [0] * n
            vec[i] = gc[i]
            d = self.nc.sync.drain()
            wait_clock.add_sem_waits(d.ins, ScopedClock({None: VectorClock(vec)}))

        self.nc.all_engine_barrier()
        assert self.sems is not None
        popped = self.nc._tile_sem_poison_stack.pop()
        assert popped is self._sem_poison
        self.nc.clear_and_free_semaphores(list(self.sems.allocated().values()))

B = 256
D = 2048
S = 8
OUT = 1000
P = 128
KO = D // P          # 16 k-tiles of 128
NT = 500             # psum n-tile (<= 512 fp32 / bank), 2 tiles cover OUT
CH = 4               # k-tiles per W DMA chunk (4 * 128 * 1000 * 4B = 2 MB)

TRACE = False        # set by test harness to collect an NTFF profile
LAST_RESULTS = None  # BassKernelResults of the most recent run

_nc_cache = {}


def _build(C):
    """Per-core program: y[C, OUT] = xT.T @ w + bias, all fp32.

    xT   : [P, KO+1, C]  xT[p, ko, c] = x_subject[c, ko*P + p] for
                         ko < KO; the last slot is all-ones (bias row).
    w    : [D, OUT]      natural layout of one subject's weights
    bias : [1, OUT]      the subject's bias row
    """
    nc = bass.Bass(enable_partition_id=False)
    xT = nc.dram_tensor("xT", [P, KO + 1, C], mybir.dt.float32r, kind="ExternalInput")
    w = nc.dram_tensor("w", [D, OUT], mybir.dt.float32r, kind="ExternalInput")
    bias = nc.dram_tensor("bias", [1, OUT], mybir.dt.float32r, kind="ExternalInput")
    y = nc.dram_tensor("y", [C, OUT], mybir.dt.float32, kind="ExternalOutput")

    w_t = w.rearrange("(ko p) n -> p ko n", p=P)  # [128, 16, 1000]
    n_chunks = KO // CH
    m_tiles = [(m0, min(P, C - m0)) for m0 in range(0, C, P)]

    with _SingleWaitTileContext(nc) as tc:
        with (
            tc.tile_pool(name="wpool", bufs=n_chunks) as wpool,
            tc.tile_pool(name="xpool", bufs=1) as xpool,
            tc.tile_pool(name="bpool", bufs=1) as bpool,
            tc.tile_pool(name="opool", bufs=4) as opool,
            tc.tile_pool(name="psum", bufs=1, space="PSUM") as psum_pool,
        ):
            # Exactly 8 DMAs total (x, bias, 4 w chunks, 2 y writes for
            # C <= 128) — one per HWDGE completion-sem lane, so no
            # instruction ever needs a lane-reuse wait.
            x_tile = xpool.tile([P, KO + 1, C], mybir.dt.float32r)
            nc.sync.dma_start(x_tile[:], xT[:])
            b_tile = bpool.tile([1, OUT], mybir.dt.float32r)
            nc.sync.dma_start(b_tile[:], bias[:])

            w_tiles = []
            for ch in range(n_chunks):
                wt = wpool.tile([P, CH, OUT], mybir.dt.float32r)
                nc.sync.dma_start(wt[:], w_t[:, ch * CH : (ch + 1) * CH, :])
                w_tiles.append(wt)

            psums = {
                (mi, n): psum_pool.tile(
                    [mc, NT], mybir.dt.float32, name=f"psum_{mi}_{n}"
                )
                for mi, (m0, mc) in enumerate(m_tiles)
                for n in range(2)
            }
            # Each instruction has ONE wait slot. The first PE op below
            # absorbs the x-DMA wait (its lhsT is a preamble constant, so
            # it has no other dependency); the bias matmuls then only
            # wait on the bias DMA, and each k-chunk's first matmul only
            # on that chunk's DMA.
            warm = psum_pool.tile([1, C], mybir.dt.float32, name="warm")
            nc.tensor.matmul(
                warm[:, :],
                nc.const_aps.tensor(0.0, [P, 1], mybir.dt.float32),
                x_tile[:, 0, :].bitcast(mybir.dt.float32),
                start=True,
                stop=True,
            )
            # Open each accumulation group with the rank-1 bias update:
            # ones[1, mc].T @ bias[1, NT].
            for mi, (m0, mc) in enumerate(m_tiles):
                for n in range(2):
                    nc.tensor.matmul(
                        psums[(mi, n)][:, :],
                        x_tile[0:1, KO, m0 : m0 + mc],
                        b_tile[0:1, n * NT : (n + 1) * NT],
                        start=True,
                        stop=False,
                    )
            # k-contiguous loop: each W chunk is consumed for every
            # (m, n) output tile as soon as it lands, then is dead.
            for ko in range(KO):
                wt = w_tiles[ko // CH]
                for mi, (m0, mc) in enumerate(m_tiles):
                    lhsT = x_tile[:, ko, m0 : m0 + mc]
                    for n in range(2):
                        nc.tensor.matmul(
                            psums[(mi, n)][:, :],
                            lhsT,
                            wt[:, ko % CH, n * NT : (n + 1) * NT],
                            start=False,
                            stop=(ko == KO - 1),
                        )
            for mi, (m0, mc) in enumerate(m_tiles):
                for n in range(2):
                    ot = opool.tile([mc, NT], mybir.dt.float32)
                    nc.vector.tensor_copy(ot[:], psums[(mi, n)][:])
                    nc.sync.dma_start(y[m0 : m0 + mc, n * NT : (n + 1) * NT], ot[:])
    return nc


def _capacity(max_count):
    c = 64
    while c < max_count:
        c *= 2
    return c


def kernel(x, subject_ids, W, b):
    global LAST_RESULTS
    x = np.ascontiguousarray(np.asarray(x, dtype=np.float32))
    sid = np.asarray(subject_ids).astype(np.int64)
    W = np.ascontiguousarray(np.asarray(W, dtype=np.float32))
    b = np.ascontiguousarray(np.asarray(b, dtype=np.float32))

    groups = [np.nonzero(sid == s)[0] for s in range(S)]
    C = _capacity(max((len(g) for g in groups), default=1))

    if C not in _nc_cache:
        _nc_cache[C] = _build(C)
    nc = _nc_cache[C]

    in_maps = []
    for s in range(S):
        idx = groups[s]
        xs = np.zeros((C, D), dtype=np.float32)
        xs[: len(idx)] = x[idx]
        # [p, ko, c] = xs[c, ko*P + p]; extra all-ones k-slot for bias
        xT = np.empty((P, KO + 1, C), dtype=np.float32)
        xT[:, :KO, :] = xs.T.reshape(KO, P, C).transpose(1, 0, 2)
        xT[:, KO, :] = 1.0
        in_maps.append({"xT": xT, "w": W[s], "bias": b[s : s + 1]})

    LAST_RESULTS = run_bass_kernel_spmd(
        nc, in_maps, core_ids=list(range(S)), trace=TRACE
    )

    out = np.zeros((B, OUT), dtype=np.float32)
    for s in range(S):
        idx = groups[s]
        out[idx] = LAST_RESULTS.results[s]["y"][: len(idx)]
    return out
